# revision 47
# baseline (speedup 1.0000x reference)
"""Llama attention layer on 8 Trainium2 NeuronCores (tensor-parallel over heads).

Sharding: each core owns 2 of 16 heads. wq/wk/wv column-sharded, wo row-sharded.
x is replicated; the o_proj partial outputs are summed on the host (the
"all-reduce" of the row-parallel output).

On-device layout is fully transposed ("feature-major") so that no transposes
are needed anywhere:
  - xT        [d, tok]      d on partitions
  - qT, kT    [j', tok]     j' = per-head feature, parity-major (RoPE perm)
  - scoresT   [t, s]        from matmul(lhsT=kT tile, rhs=qT tile)
  - expT      [t, s]        exp on ACT; causal mask = multiply by exp(mask)
  - outT      [j, s]        from matmul(lhsT=v tile [t, j], rhs=expT)
  - y         [s, e]        from matmul(lhsT=outT tile, rhs=woT)
No max-subtraction: |scores| is O(5) for this distribution and exp is
computed in fp32 from the fp32 psum.

Performance structure (what got this from 532us to ~386us):
  - Interleaved emission: attention blocks (ACT-bound on exp) are emitted
    between projection token-blocks and o_proj s-tiles (pure PE work), so
    the in-order PE stream never starves while ACT runs.  Per-block norms
    are staggered one block behind; o_proj tiles are emitted as soon as
    their s-block is normalized on both heads.
  - Causal triangle: fully-masked t-tiles are skipped; diagonal tiles only
    compute the live [o:512] column slice in score/exp/av/den; the mask
    multiply shrinks to one [128,128] triangle chunk.
  - Paired ops: score/proj/o_proj matmuls write 2-bank [128, 2x512] psum
    tiles so exp and psum evictions run as single double-width ops (halves
    per-op overhead and semaphore traffic).
  - Softmax denominator: per-tile [1,512] ones-matmuls are column-tiled
    (tile_position) into 4 concurrent PE column groups (M=32 redundant
    rows); a selector matmul (rows 0/32/64/96) sums + broadcasts them,
    reciprocal_approx_fast on DVE, and the 1/den multiply is folded into
    the av psum->sbuf eviction.
  - Input DMAs are sliced per 128-row chunk and interleaved (weights on the
    sync queue, x on the scalar queue) so the first matmuls unblock after
    ~200KB instead of ~9MB.
"""

import math
import os

import numpy as np
import ml_dtypes

import concourse.bass as bass
import concourse.tile as tile
from concourse import bacc, mybir
from concourse.bass_utils import run_bass_kernel_spmd
from contextlib import ExitStack

BF16 = mybir.dt.bfloat16
F32 = mybir.dt.float32
AF = mybir.ActivationFunctionType

N_CORES = 8
B, S, D = 2, 2048, 2048
H = 16                      # total heads
HPC = H // N_CORES          # heads per core = 2
HD = D // H                 # head dim = 128
EC = HPC * HD               # features per core = 256
TOK = B * S                 # 4096
P = 128
NDT = D // P                # 16 d-tiles
NTB = TOK // 512            # 8 tok blocks of 512
NSB = S // 512              # 4 s-blocks per batch
NTT = S // P                # 16 t-tiles per batch
SCALE = 1.0 / math.sqrt(HD)

ts = bass.ts
ds = bass.ds

LAST_EXEC_NS = None
TRACE = bool(int(os.environ.get("KERNEL_TRACE", "0")))
BACKEND = os.environ.get("KERNEL_BACKEND", "hw")  # "hw" | "sim"

_PROGRAM_CACHE = {}


def _install_trace_hook():
    """Register an NTFF-profile hook for trace=True under axon when the
    image's antenv lacks axon_hooks (replicates trn_boot's ctypes shim)."""
    import sys as _sys
    import types
    import ctypes
    import contextlib

    try:
        from antenv.axon_hooks import get_axon_ntff_profile_hook  # noqa: F401
        return True
    except ImportError:
        pass

    so_path = "/opt/axon/libaxon_pjrt.so"
    if not os.path.exists(so_path):
        return False
    lib = ctypes.CDLL(so_path)
    if not hasattr(lib, "axon_start_nrt_profile"):
        return False
    lib.axon_start_nrt_profile.argtypes = [
        ctypes.POINTER(ctypes.c_int64),
        ctypes.c_size_t,
    ]
    lib.axon_start_nrt_profile.restype = ctypes.c_int64
    lib.axon_stop_nrt_profile.argtypes = [ctypes.c_char_p]
    lib.axon_stop_nrt_profile.restype = ctypes.c_int64

    @contextlib.contextmanager
    def _hook(output_dir, device_ids):
        import jax
        jax.devices()
        if device_ids:
            ids = (ctypes.c_int64 * len(device_ids))(*device_ids)
            rc = lib.axon_start_nrt_profile(ids, len(device_ids))
        else:
            rc = lib.axon_start_nrt_profile(None, 0)
        if rc != 0:
            raise RuntimeError(f"axon_start_nrt_profile rc={rc}")
        try:
            yield
        finally:
            n = lib.axon_stop_nrt_profile(str(output_dir).encode())
            print(f"profile: {n} file(s) written to {output_dir}")

    import antenv
    mod = types.ModuleType("antenv.axon_hooks")
    mod._hook = _hook
    mod.get_axon_ntff_profile_hook = lambda: _hook
    mod.set_axon_ntff_profile_hook = lambda h: None
    _sys.modules["antenv.axon_hooks"] = mod
    antenv.axon_hooks = mod

    # artifact upload has no bucket access in this container; stub it
    import concourse.bass_utils as _bu
    _bu.upload_artifacts = lambda tmpdir: f"local://{tmpdir}"
    return True


def _classify_mask(mask):
    """Split the [S, S] additive mask into [t-128 x s-512] blocks per s-block.

    Returns (blocks, fpats, cpats): blocks[m] = list of (j, o, fpid, cpid)
    t-tiles for s-block m, where o is the column offset (multiple of 128) of
    the first live query column (only the [o:512] slice is computed), fpid
    indexes a full [128, 512] exp(mask) pattern, and cpid indexes a [128, 128]
    chunk pattern applied to columns [o:o+128).  A tile uses at most one of
    fpid/cpid.  Tiles in the first round of 4 (idx < 4) are forced to o=0 so
    every den column-group's start=True matmul covers the full width.
    """
    mm = np.asarray(mask, np.float32).reshape(S, S)
    fpats, fids = [], {}
    cpats, cids = [], {}
    blocks = []
    for m in range(NSB):
        lst = []
        for j in range(NTT):
            blk = mm[m * 512:(m + 1) * 512, j * P:(j + 1) * P]  # [s, t]
            if np.all(blk <= -30.0):
                continue  # exp == 0: contributes nothing to av or den
            idx = len(lst)
            if np.all(blk == 0.0):
                lst.append((j, 0, None, None))
                continue
            pt = np.exp(np.minimum(blk.T, 80.0)).astype(np.float32)  # [t, s]
            # leading all-zero 128-col chunks can be skipped entirely
            o = 0
            while o + P <= 512 and np.all(pt[:, o:o + P] == 0.0):
                o += P
            # chunk-pattern eligible: everything right of the chunk is ones,
            # and a sliced (o>0) tile must not be the start=True matmul of its
            # den column group (idx>=4 for quad mode; idx>=1 for the serial
            # single-group mode used when the block has only 4 tiles).
            ok_slice = (o == 0) or idx >= 4 or (m == 0 and idx >= 1)
            sliceable = ok_slice and np.all(pt[:, o + P:] == 1.0)
            if sliceable:
                ch = np.ascontiguousarray(pt[:, o:o + P])
                key = ch.tobytes()
                if key not in cids:
                    cids[key] = len(cpats)
                    cpats.append(ch)
                lst.append((j, o, None, cids[key]))
            else:
                key = pt.tobytes()
                if key not in fids:
                    fids[key] = len(fpats)
                    fpats.append(pt)
                lst.append((j, 0, fids[key], None))
        blocks.append(lst)
    return blocks, fpats, cpats


def _emit(ctx, tc, io, blocks, npatf, npatc):
    nc = tc.nc

    const = ctx.enter_context(tc.tile_pool(name="const", bufs=1))
    persist = ctx.enter_context(tc.tile_pool(name="persist", bufs=1))
    xt_pool = ctx.enter_context(tc.tile_pool(name="xt_pool", bufs=3))
    rope_pool = ctx.enter_context(tc.tile_pool(name="rope_pool", bufs=2))
    exp_pool = ctx.enter_context(tc.tile_pool(name="exp_pool", bufs=10))
    den_pool = ctx.enter_context(tc.tile_pool(name="den_pool", bufs=4))
    recip_pool = ctx.enter_context(tc.tile_pool(name="recip_pool", bufs=2))
    y_pool = ctx.enter_context(tc.tile_pool(name="y_pool", bufs=3))
    psum_big = ctx.enter_context(tc.tile_pool(name="psum_big", bufs=2, space="PSUM"))
    psum_acc = ctx.enter_context(tc.tile_pool(name="psum_acc", bufs=2, space="PSUM"))
    psum_den = ctx.enter_context(tc.tile_pool(name="psum_den", bufs=1, space="PSUM"))
    psum_bc = ctx.enter_context(tc.tile_pool(name="psum_bc", bufs=1, space="PSUM"))

    # --- constants / weights ---
    # DMA order is the sync queue order: interleave the first weight tile
    # with the first two token blocks (sliced per 128-row chunk) so the very
    # first projection matmuls unblock after ~200KB instead of ~9MB.
    wq_sb = const.tile([P, NDT, HPC, P], BF16)
    wk_sb = const.tile([P, NDT, HPC, P], BF16)
    wv_sb = const.tile([P, NDT, EC], BF16)
    wo_sb = const.tile([P, HPC, D], BF16)
    cos_sb = const.tile([P, TOK], BF16)
    sin_sb = const.tile([P, TOK], BF16)
    patf_sb = const.tile([P, npatf, 512], BF16)
    patc_sb = const.tile([P, npatc, P], BF16)
    xt_tiles = {}
    for tb in (0, 1):
        xt_tiles[tb] = xt_pool.tile([P, NDT, 512], BF16, tag="xt", name=f"xt_t{tb}")
    for d2 in range(NDT // 2):
        nc.sync.dma_start(wq_sb[:, 2 * d2:2 * d2 + 2], io["wqt"][:, 2 * d2:2 * d2 + 2])
        nc.scalar.dma_start(xt_tiles[0][:, 2 * d2:2 * d2 + 2, :],
                            io["xt"][:, 2 * d2:2 * d2 + 2, ts(0, 512)])
    for d2 in range(NDT // 2):
        nc.sync.dma_start(wk_sb[:, 2 * d2:2 * d2 + 2], io["wkt"][:, 2 * d2:2 * d2 + 2])
        nc.scalar.dma_start(xt_tiles[1][:, 2 * d2:2 * d2 + 2, :],
                            io["xt"][:, 2 * d2:2 * d2 + 2, ts(1, 512)])
    for d4 in range(NDT // 4):
        nc.sync.dma_start(wv_sb[:, 4 * d4:4 * d4 + 4], io["wvt"][:, 4 * d4:4 * d4 + 4])
    nc.sync.dma_start(cos_sb[:], io["cos2"][:])
    nc.sync.dma_start(sin_sb[:], io["sin2"][:])
    ones32 = const.tile([P, 32], BF16)
    nc.any.memset(ones32[:], 1.0)
    # selector: picks one representative row per 32-row column group and
    # sums them (bcast matmul lhsT), zeroing the redundant copies.
    sel4 = const.tile([P, P], BF16)
    nc.any.memset(sel4[:], 0.0)
    for g in range(4):
        nc.any.memset(sel4[ds(32 * g, 1), :], 1.0)
    sel1 = const.tile([P, P], BF16)
    nc.any.memset(sel1[:], 0.0)
    nc.any.memset(sel1[ds(0, 1), :], 1.0)

    nc.sync.dma_start(patf_sb[:], io["patf"][:])
    nc.sync.dma_start(patc_sb[:], io["patc"][:])
    nc.sync.dma_start(wo_sb[:], io["wot"][:])

    q_sb = persist.tile([P, HPC, TOK], BF16)   # [parity*64+i, h, tok]
    k_sb = persist.tile([P, HPC, TOK], BF16)
    v_sb = persist.tile([P, TOK // P, EC], BF16)  # [t%128, t-tile, (h, j)]
    outT_sb = persist.tile([P, B * HPC, S], BF16)  # [j, pair, s]

    # PE warm-up: back-to-back tiny matmuls with no data deps, filling the
    # startup DMA window so the HAM clock-gate opens before real work lands.
    warm_ps = psum_bc.tile([P, 512], F32, tag="bc")
    for w in range(12):
        nc.tensor.matmul(warm_ps[:, ds(0, 16)], lhsT=sel4[:], rhs=sel4[:, 0:16],
                         start=True, stop=True)

    den_rows = {pi: {} for pi in range(B * HPC)}

    def emit_proj(tb):
        """q/k/v projections + RoPE for one 512-token block."""
        if tb in xt_tiles:
            xt_t = xt_tiles.pop(tb)
        else:
            xt_t = xt_pool.tile([P, NDT, 512], BF16, tag="xt")
            for d4 in range(NDT // 4):
                nc.sync.dma_start(xt_t[:, 4 * d4:4 * d4 + 4, :],
                                  io["xt"][:, 4 * d4:4 * d4 + 4, ts(tb, 512)])

        # RoPE operand broadcast: both heads in one DVE op via a 0-stride
        # broadcast of cos/sin over h; each projection's rope chain is
        # emitted right after its own eviction so the DVE finishes roped
        # q/k as early as possible (score LDWs wait on it).
        cos_b = cos_sb[:, ts(tb, 512)].unsqueeze(1).broadcast_to([P, HPC, 512])
        sin_b = sin_sb[:, ts(tb, 512)].unsqueeze(1).broadcast_to([P, HPC, 512])
        for w_sb, dst in ((wq_sb, q_sb), (wk_sb, k_sb)):
            qk_ps = psum_big.tile([P, HPC, 512], F32, tag="big", name="qk_ps")
            for h in range(HPC):
                for dt in range(NDT):
                    nc.tensor.matmul(
                        qk_ps[:, h, :], lhsT=w_sb[:, dt, h, :], rhs=xt_t[:, dt, :],
                        start=(dt == 0), stop=(dt == NDT - 1),
                    )
            nc.vector.tensor_copy(dst[:, :, ts(tb, 512)], qk_ps[:])
            swp = rope_pool.tile([P, HPC, 512], BF16, tag="swp")
            nc.sync.dma_start(swp[0:64, :, :], dst[64:128, :, ts(tb, 512)])
            nc.sync.dma_start(swp[64:128, :, :], dst[0:64, :, ts(tb, 512)])
            r1 = rope_pool.tile([P, HPC, 512], BF16, tag="r1")
            nc.vector.tensor_mul(r1[:], dst[:, :, ts(tb, 512)], cos_b)
            r2 = rope_pool.tile([P, HPC, 512], BF16, tag="r2")
            nc.vector.tensor_mul(r2[:], swp[:], sin_b)
            nc.vector.tensor_add(dst[:, :, ts(tb, 512)], r1[:], r2[:])

        v_ps = psum_big.tile([P, 4, EC], F32, tag="big", name="v_ps")
        for q4 in range(4):
            for dt in range(NDT):
                nc.tensor.matmul(
                    v_ps[:, q4, :], lhsT=xt_t[:, dt, ts(q4, P)], rhs=wv_sb[:, dt, :],
                    start=(dt == 0), stop=(dt == NDT - 1),
                )
        nc.vector.tensor_copy(v_sb[:, tb * 4:(tb + 1) * 4, :], v_ps[:])

    def emit_att(pi, m):
        """scores + exp + av/den matmuls for pair pi, 512-query block m."""
        b, h = divmod(pi, HPC)
        s_sl = ds(b * S + m * 512, 512)
        tlist = blocks[m]
        n_t = len(tlist)
        av_ps = psum_acc.tile([P, 512], F32, tag="acc")
        den4_ps = psum_den.tile([P, 512], F32, tag="den")
        # scores + exp first (PE runs ahead of ACT through the psum
        # pool); den/av matmuls afterwards so PE never waits on exp.
        # Diagonal tiles only compute the live [o:512] column slice.
        # Tiles are processed in pairs sharing a 2-bank psum tile so a pair
        # of full tiles needs a single (double-width) exp activation.
        exs = []  # per tile: (ex2 tile, half index)
        for p2 in range(n_t // 2):
            pair = tlist[2 * p2:2 * p2 + 2]
            sc_ps = psum_big.tile([P, 2, 512], F32, tag="big", name="sc_ps")
            ex2 = exp_pool.tile([P, 2, 512], BF16, tag="ex2")
            for hh, (j, o, fpid, cpid) in enumerate(pair):
                nc.tensor.matmul(
                    sc_ps[:, hh, ds(o, 512 - o)],
                    lhsT=k_sb[:, h, ds(b * S + j * P, P)],
                    rhs=q_sb[:, h, ds(b * S + m * 512 + o, 512 - o)],
                    start=True, stop=True,
                )
            if all(o == 0 for _, o, _, _ in pair):
                nc.scalar.activation(ex2[:], sc_ps[:], AF.Exp, scale=SCALE)
            else:
                for hh, (j, o, fpid, cpid) in enumerate(pair):
                    c_sl = ds(o, 512 - o)
                    nc.scalar.activation(ex2[:, hh, c_sl], sc_ps[:, hh, c_sl],
                                         AF.Exp, scale=SCALE)
            for hh, (j, o, fpid, cpid) in enumerate(pair):
                if fpid is not None:
                    nc.vector.tensor_mul(ex2[:, hh, :], ex2[:, hh, :],
                                         patf_sb[:, fpid, :])
                elif cpid is not None:
                    nc.vector.tensor_mul(ex2[:, hh, ds(o, P)],
                                         ex2[:, hh, ds(o, P)],
                                         patc_sb[:, cpid, :])
                exs.append((ex2, hh))
        # av matmuls per round of 4, then the 4 den matmuls issued
        # back-to-back into distinct PE column groups so they execute
        # concurrently (~1 matmul's time for all 4).
        single_group = (n_t == 4)
        nr = n_t // 4
        for r in range(nr):
            for q4 in range(4):
                idx = 4 * r + q4
                j, o = tlist[idx][0], tlist[idx][1]
                c_sl = ds(o, 512 - o)
                ex2, hh = exs[idx]
                nc.tensor.matmul(
                    av_ps[:, c_sl], lhsT=v_sb[:, b * NTT + j, ds(h * HD, HD)],
                    rhs=ex2[:, hh, c_sl],
                    start=(idx == 0), stop=(idx == n_t - 1),
                )
            for q4 in range(4):
                idx = 4 * r + q4
                o = tlist[idx][1]
                c_sl = ds(o, 512 - o)
                ex2, hh = exs[idx]
                if single_group:
                    # sliced tiles may appear in the first round of 4 only
                    # if they all accumulate into one column group whose
                    # first (idx 0) matmul is full width
                    nc.tensor.matmul(
                        den4_ps[ds(0, 32), c_sl], lhsT=ones32[:],
                        rhs=ex2[:, hh, c_sl],
                        start=(idx == 0), stop=(idx == n_t - 1),
                    )
                else:
                    nc.tensor.matmul(
                        den4_ps[ds(32 * q4, 32), c_sl], lhsT=ones32[:],
                        rhs=ex2[:, hh, c_sl], start=(r == 0), stop=(r == nr - 1),
                        tile_position=(0, 32 * q4),
                    )
        # evict the denominator; the av psum is held (psum_acc has 2 bufs)
        # until the staggered norm step, which folds the 1/den multiply into
        # the psum->sbuf eviction itself.
        den4_sb = den_pool.tile([P, 512], BF16, tag="denr")
        nc.vector.tensor_copy(den4_sb[:], den4_ps[:])
        den_rows[pi][m] = (den4_sb, sel1 if single_group else sel4, av_ps)

    def emit_norm(pi, m2):
        """broadcast den, reciprocal, then evict av psum * (1/den) -> outT."""
        d4, sel, av_ps = den_rows[pi].pop(m2)
        bc_ps = psum_bc.tile([P, 512], F32, tag="bc")
        nc.tensor.matmul(bc_ps[:], lhsT=sel[:], rhs=d4[:],
                         start=True, stop=True)
        rc = recip_pool.tile([P, 512], F32, tag="rc")
        nc.vector.reciprocal_approx_fast(rc[:], bc_ps[:])
        sl2 = ds(m2 * 512, 512)
        nc.vector.tensor_mul(outT_sb[:, pi, sl2], av_ps[:], rc[:])

    def emit_oproj(st):
        """o_proj partial y[s, e] = sum_h outT_h^T @ woT_h for one s-tile."""
        b = st // NTT
        sl = st % NTT
        for e2 in range(D // 1024):
            y_ps = psum_big.tile([P, 1024], F32, tag="big", name="y_ps")
            for i2 in range(2):
                for h in range(HPC):
                    nc.tensor.matmul(
                        y_ps[:, ds(i2 * 512, 512)],
                        lhsT=outT_sb[:, b * HPC + h, ts(sl, P)],
                        rhs=wo_sb[:, h, ts(e2 * 2 + i2, 512)],
                        start=(h == 0), stop=(h == HPC - 1),
                    )
            y_sb = y_pool.tile([P, 1024], BF16, tag="y")
            if (st * 2 + e2) % 2 == 0:
                nc.scalar.copy(y_sb[:], y_ps[:])
            else:
                nc.vector.tensor_copy(y_sb[:], y_ps[:])
            nc.sync.dma_start(io["y"][st, :, ts(e2, 1024)], y_sb[:])

    # Interleaved schedule: attention blocks (ACT-heavy: exp) are emitted
    # between projection / o_proj chunks (pure PE) so the in-order PE stream
    # always has matmul work while the scalar engine chews on exponentials.
    # Per-block norms are staggered one block behind their attention so the
    # chain (bcast -> recip -> mul) never gates the tail; o_proj tiles are
    # emitted progressively as soon as their s-block is normalized on both
    # heads (OPROJ(st) needs NORM(*, (st % 16) // 4)).
    emit_proj(0)
    emit_proj(1)
    emit_proj(2)
    emit_att(0, 0)
    emit_att(1, 0)
    emit_proj(3)
    emit_norm(0, 0)
    emit_norm(1, 0)
    emit_att(0, 1)
    emit_att(1, 1)
    emit_proj(4)
    emit_norm(0, 1)
    emit_norm(1, 1)
    emit_att(0, 2)
    emit_att(1, 2)
    emit_proj(5)
    emit_norm(0, 2)
    emit_norm(1, 2)
    emit_att(0, 3)
    emit_att(1, 3)
    emit_proj(6)
    emit_norm(0, 3)
    emit_norm(1, 3)
    emit_att(2, 1)
    emit_att(3, 1)
    emit_proj(7)
    emit_norm(2, 1)
    emit_norm(3, 1)
    emit_att(2, 2)
    emit_att(3, 2)
    for st in range(0, 4):
        emit_oproj(st)
    emit_norm(2, 2)
    emit_norm(3, 2)
    emit_att(2, 3)
    emit_att(3, 3)
    for st in range(4, 16):
        emit_oproj(st)
    emit_norm(2, 3)
    emit_norm(3, 3)
    emit_att(2, 0)
    emit_att(3, 0)
    for st in range(20, 26):
        emit_oproj(st)
    emit_norm(2, 0)
    emit_norm(3, 0)
    for st in range(26, 32):
        emit_oproj(st)
    for st in range(16, 20):
        emit_oproj(st)


def _build_program(blocks_key, blocks, npatf, npatc):
    nc = bacc.Bacc(
        "TRN2", target_bir_lowering=False, debug=False, enable_asserts=False
    )
    io = {
        "xt": nc.dram_tensor("xt", [P, NDT, TOK], BF16, kind="ExternalInput").ap(),
        "wqt": nc.dram_tensor("wqt", [P, NDT, HPC, P], BF16, kind="ExternalInput").ap(),
        "wkt": nc.dram_tensor("wkt", [P, NDT, HPC, P], BF16, kind="ExternalInput").ap(),
        "wvt": nc.dram_tensor("wvt", [P, NDT, EC], BF16, kind="ExternalInput").ap(),
        "wot": nc.dram_tensor("wot", [P, HPC, D], BF16, kind="ExternalInput").ap(),
        "cos2": nc.dram_tensor("cos2", [P, TOK], BF16, kind="ExternalInput").ap(),
        "sin2": nc.dram_tensor("sin2", [P, TOK], BF16, kind="ExternalInput").ap(),
        "patf": nc.dram_tensor("patf", [P, npatf, 512], BF16, kind="ExternalInput").ap(),
        "patc": nc.dram_tensor("patc", [P, npatc, P], BF16, kind="ExternalInput").ap(),
        "y": nc.dram_tensor("y", [TOK // P, P, D], BF16, kind="ExternalOutput").ap(),
    }
    with tile.TileContext(nc) as tc:
        with ExitStack() as ctx:
            _emit(ctx, tc, io, blocks, npatf, npatc)
    nc.compile()
    return nc


def _get_program(mask):
    blocks, fpats, cpats = _classify_mask(mask)
    key = tuple(tuple(b) for b in blocks)
    if key not in _PROGRAM_CACHE:
        npatf = max(len(fpats), 1)
        npatc = max(len(cpats), 1)
        nc = _build_program(key, blocks, npatf, npatc)
        _PROGRAM_CACHE[key] = (nc, npatf, npatc)
    nc, npatf, npatc = _PROGRAM_CACHE[key]
    patf_np = np.zeros((P, npatf, 512), np.float32)
    for i, pt in enumerate(fpats):
        patf_np[:, i, :] = pt
    patc_np = np.zeros((P, npatc, P), np.float32)
    for i, pt in enumerate(cpats):
        patc_np[:, i, :] = pt
    return nc, patf_np, patc_np


def _bf16(a):
    return np.asarray(a, np.float32).astype(ml_dtypes.bfloat16)


def kernel(x, wq, wk, wv, wo, freqs_cos, freqs_sin, mask):
    global LAST_EXEC_NS
    x = np.asarray(x, np.float32)
    wq = np.asarray(wq, np.float32)
    wk = np.asarray(wk, np.float32)
    wv = np.asarray(wv, np.float32)
    wo = np.asarray(wo, np.float32)
    freqs_cos = np.asarray(freqs_cos, np.float32)
    freqs_sin = np.asarray(freqs_sin, np.float32)

    nc, patf_np, patc_np = _get_program(mask)

    # xT: [d, tok] -> [dp, dt, tok]
    xt = _bf16(
        x.reshape(TOK, D).T.reshape(NDT, P, TOK).transpose(1, 0, 2)
    )

    # cos/sin, parity-major RoPE operands: [128, tok]
    cosT = np.tile(freqs_cos.T, (1, B))          # [64, TOK]
    sinT = np.tile(freqs_sin.T, (1, B))
    cos2 = _bf16(np.concatenate([cosT, cosT], axis=0))
    sin2 = _bf16(np.concatenate([-sinT, sinT], axis=0))
    patf = _bf16(patf_np)
    patc = _bf16(patc_np)

    # per-head parity-major row permutation for q/k weights
    perm1 = np.r_[np.arange(0, P, 2), np.arange(1, P, 2)]

    in_maps = []
    for c in range(N_CORES):
        rows = slice(c * EC, (c + 1) * EC)
        wq_c, wk_c, wv_c = wq[rows], wk[rows], wv[rows]   # [256, D]
        wo_c = wo[:, rows]                                # [D, 256]
        row_perm = np.concatenate([h * P + perm1 for h in range(HPC)])
        wqt = _bf16(wq_c[row_perm].T.reshape(NDT, P, HPC, P).transpose(1, 0, 2, 3))
        wkt = _bf16(wk_c[row_perm].T.reshape(NDT, P, HPC, P).transpose(1, 0, 2, 3))
        wvt = _bf16(wv_c.T.reshape(NDT, P, EC).transpose(1, 0, 2))
        wot = _bf16(wo_c.T.reshape(HPC, P, D).transpose(1, 0, 2))
        in_maps.append({
            "xt": xt, "wqt": wqt, "wkt": wkt, "wvt": wvt, "wot": wot,
            "cos2": cos2, "sin2": sin2, "patf": patf, "patc": patc,
        })

    if BACKEND == "sim":
        from concourse.bass_interp import CoreSim
        results = []
        for c in range(N_CORES):
            sim = CoreSim(nc, trace=False)
            for name, arr in in_maps[c].items():
                sim.tensor(name)[:] = arr
            sim.tensor("y")[:] = 0
            sim.simulate()
            results.append({"y": np.array(sim.tensor("y"))})
    else:
        do_trace = TRACE and _install_trace_hook()
        res = run_bass_kernel_spmd(
            nc, in_maps, core_ids=list(range(N_CORES)), trace=do_trace,
        )
        results = res.results
        LAST_EXEC_NS = res.exec_time_ns

    y = np.zeros((TOK, D), np.float32)
    for c in range(N_CORES):
        y += results[c]["y"].reshape(TOK, D).astype(np.float32)
    return y.reshape(B, S, D)



# revision 49
# speedup vs baseline: 1.0146x; 1.0146x over previous
"""Llama attention layer on 8 Trainium2 NeuronCores (tensor-parallel over heads).

Sharding: each core owns 2 of 16 heads. wq/wk/wv column-sharded, wo row-sharded.
x is replicated; the o_proj partial outputs are summed on the host (the
"all-reduce" of the row-parallel output).

On-device layout is fully transposed ("feature-major") so that no transposes
are needed anywhere:
  - xT        [d, tok]      d on partitions
  - qT, kT    [j', tok]     j' = per-head feature, parity-major (RoPE perm)
  - scoresT   [t, s]        from matmul(lhsT=kT tile, rhs=qT tile)
  - expT      [t, s]        exp on ACT; causal mask = multiply by exp(mask)
  - outT      [j, s]        from matmul(lhsT=v tile [t, j], rhs=expT)
  - y         [s, e]        from matmul(lhsT=outT tile, rhs=woT)
No max-subtraction: |scores| is O(5) for this distribution and exp is
computed in fp32 from the fp32 psum.

Performance structure (what got this from 532us to ~386us):
  - Interleaved emission: attention blocks (ACT-bound on exp) are emitted
    between projection token-blocks and o_proj s-tiles (pure PE work), so
    the in-order PE stream never starves while ACT runs.  Per-block norms
    are staggered one block behind; o_proj tiles are emitted as soon as
    their s-block is normalized on both heads.
  - Causal triangle: fully-masked t-tiles are skipped; diagonal tiles only
    compute the live [o:512] column slice in score/exp/av/den; the mask
    multiply shrinks to one [128,128] triangle chunk.
  - Paired ops: score/proj/o_proj matmuls write 2-bank [128, 2x512] psum
    tiles so exp and psum evictions run as single double-width ops (halves
    per-op overhead and semaphore traffic).
  - Softmax denominator: per-tile [1,512] ones-matmuls are column-tiled
    (tile_position) into 4 concurrent PE column groups (M=32 redundant
    rows); a selector matmul (rows 0/32/64/96) sums + broadcasts them,
    reciprocal_approx_fast on DVE, and the 1/den multiply is folded into
    the av psum->sbuf eviction.
  - Input DMAs are sliced per 128-row chunk and interleaved (weights on the
    sync queue, x on the scalar queue) so the first matmuls unblock after
    ~200KB instead of ~9MB.
"""

import math
import os

import numpy as np
import ml_dtypes

import concourse.bass as bass
import concourse.tile as tile
from concourse import bacc, mybir
from concourse.bass_utils import run_bass_kernel_spmd
from contextlib import ExitStack

BF16 = mybir.dt.bfloat16
F32 = mybir.dt.float32
AF = mybir.ActivationFunctionType

N_CORES = 8
B, S, D = 2, 2048, 2048
H = 16                      # total heads
HPC = H // N_CORES          # heads per core = 2
HD = D // H                 # head dim = 128
EC = HPC * HD               # features per core = 256
TOK = B * S                 # 4096
P = 128
NDT = D // P                # 16 d-tiles
NTB = TOK // 512            # 8 tok blocks of 512
NSB = S // 512              # 4 s-blocks per batch
NTT = S // P                # 16 t-tiles per batch
SCALE = 1.0 / math.sqrt(HD)

ts = bass.ts
ds = bass.ds

LAST_EXEC_NS = None
TRACE = bool(int(os.environ.get("KERNEL_TRACE", "0")))
BACKEND = os.environ.get("KERNEL_BACKEND", "hw")  # "hw" | "sim"

_PROGRAM_CACHE = {}


def _install_trace_hook():
    """Register an NTFF-profile hook for trace=True under axon when the
    image's antenv lacks axon_hooks (replicates trn_boot's ctypes shim)."""
    import sys as _sys
    import types
    import ctypes
    import contextlib

    try:
        from antenv.axon_hooks import get_axon_ntff_profile_hook  # noqa: F401
        return True
    except ImportError:
        pass

    so_path = "/opt/axon/libaxon_pjrt.so"
    if not os.path.exists(so_path):
        return False
    lib = ctypes.CDLL(so_path)
    if not hasattr(lib, "axon_start_nrt_profile"):
        return False
    lib.axon_start_nrt_profile.argtypes = [
        ctypes.POINTER(ctypes.c_int64),
        ctypes.c_size_t,
    ]
    lib.axon_start_nrt_profile.restype = ctypes.c_int64
    lib.axon_stop_nrt_profile.argtypes = [ctypes.c_char_p]
    lib.axon_stop_nrt_profile.restype = ctypes.c_int64

    @contextlib.contextmanager
    def _hook(output_dir, device_ids):
        import jax
        jax.devices()
        if device_ids:
            ids = (ctypes.c_int64 * len(device_ids))(*device_ids)
            rc = lib.axon_start_nrt_profile(ids, len(device_ids))
        else:
            rc = lib.axon_start_nrt_profile(None, 0)
        if rc != 0:
            raise RuntimeError(f"axon_start_nrt_profile rc={rc}")
        try:
            yield
        finally:
            n = lib.axon_stop_nrt_profile(str(output_dir).encode())
            print(f"profile: {n} file(s) written to {output_dir}")

    import antenv
    mod = types.ModuleType("antenv.axon_hooks")
    mod._hook = _hook
    mod.get_axon_ntff_profile_hook = lambda: _hook
    mod.set_axon_ntff_profile_hook = lambda h: None
    _sys.modules["antenv.axon_hooks"] = mod
    antenv.axon_hooks = mod

    # artifact upload has no bucket access in this container; stub it
    import concourse.bass_utils as _bu
    _bu.upload_artifacts = lambda tmpdir: f"local://{tmpdir}"
    return True


def _classify_mask(mask):
    """Split the [S, S] additive mask into [t-128 x s-512] blocks per s-block.

    Returns (blocks, fpats, cpats): blocks[m] = list of (j, o, fpid, cpid)
    t-tiles for s-block m, where o is the column offset (multiple of 128) of
    the first live query column (only the [o:512] slice is computed), fpid
    indexes a full [128, 512] exp(mask) pattern, and cpid indexes a [128, 128]
    chunk pattern applied to columns [o:o+128).  A tile uses at most one of
    fpid/cpid.  Tiles in the first round of 4 (idx < 4) are forced to o=0 so
    every den column-group's start=True matmul covers the full width.
    """
    mm = np.asarray(mask, np.float32).reshape(S, S)
    fpats, fids = [], {}
    cpats, cids = [], {}
    blocks = []
    for m in range(NSB):
        lst = []
        for j in range(NTT):
            blk = mm[m * 512:(m + 1) * 512, j * P:(j + 1) * P]  # [s, t]
            if np.all(blk <= -30.0):
                continue  # exp == 0: contributes nothing to av or den
            idx = len(lst)
            if np.all(blk == 0.0):
                lst.append((j, 0, None, None))
                continue
            pt = np.exp(np.minimum(blk.T, 80.0)).astype(np.float32)  # [t, s]
            # leading all-zero 128-col chunks can be skipped entirely
            o = 0
            while o + P <= 512 and np.all(pt[:, o:o + P] == 0.0):
                o += P
            # chunk-pattern eligible: everything right of the chunk is ones,
            # and a sliced (o>0) tile must not be the start=True matmul of its
            # den column group (idx>=4 for quad mode; idx>=1 for the serial
            # single-group mode used when the block has only 4 tiles).
            ok_slice = (o == 0) or idx >= 4 or (m == 0 and idx >= 1)
            sliceable = ok_slice and np.all(pt[:, o + P:] == 1.0)
            if sliceable:
                ch = np.ascontiguousarray(pt[:, o:o + P])
                key = ch.tobytes()
                if key not in cids:
                    cids[key] = len(cpats)
                    cpats.append(ch)
                lst.append((j, o, None, cids[key]))
            else:
                key = pt.tobytes()
                if key not in fids:
                    fids[key] = len(fpats)
                    fpats.append(pt)
                lst.append((j, 0, fids[key], None))
        blocks.append(lst)
    return blocks, fpats, cpats


def _emit(ctx, tc, io, blocks, npatf, npatc):
    nc = tc.nc

    const = ctx.enter_context(tc.tile_pool(name="const", bufs=1))
    persist = ctx.enter_context(tc.tile_pool(name="persist", bufs=1))
    xt_pool = ctx.enter_context(tc.tile_pool(name="xt_pool", bufs=3))
    rope_pool = ctx.enter_context(tc.tile_pool(name="rope_pool", bufs=2))
    exp_pool = ctx.enter_context(tc.tile_pool(name="exp_pool", bufs=10))
    den_pool = ctx.enter_context(tc.tile_pool(name="den_pool", bufs=4))
    recip_pool = ctx.enter_context(tc.tile_pool(name="recip_pool", bufs=2))
    y_pool = ctx.enter_context(tc.tile_pool(name="y_pool", bufs=3))
    psum_big = ctx.enter_context(tc.tile_pool(name="psum_big", bufs=2, space="PSUM"))
    psum_acc = ctx.enter_context(tc.tile_pool(name="psum_acc", bufs=2, space="PSUM"))
    psum_den = ctx.enter_context(tc.tile_pool(name="psum_den", bufs=1, space="PSUM"))
    psum_bc = ctx.enter_context(tc.tile_pool(name="psum_bc", bufs=1, space="PSUM"))

    # --- constants / weights ---
    # DMA order is the sync queue order: interleave the first weight tile
    # with the first two token blocks (sliced per 128-row chunk) so the very
    # first projection matmuls unblock after ~200KB instead of ~9MB.
    wq_sb = const.tile([P, NDT, HPC, P], BF16)
    wk_sb = const.tile([P, NDT, HPC, P], BF16)
    wv_sb = const.tile([P, NDT, EC], BF16)
    wo_sb = const.tile([P, HPC, D], BF16)
    cos_sb = const.tile([P, TOK], BF16)
    sin_sb = const.tile([P, TOK], BF16)
    patf_sb = const.tile([P, npatf, 512], BF16)
    patc_sb = const.tile([P, npatc, P], BF16)
    xt_tiles = {}
    for tb in (0, 1):
        xt_tiles[tb] = xt_pool.tile([P, NDT, 512], BF16, tag="xt", name=f"xt_t{tb}")
    for d2 in range(NDT // 2):
        nc.sync.dma_start(wq_sb[:, 2 * d2:2 * d2 + 2], io["wqt"][:, 2 * d2:2 * d2 + 2])
        nc.scalar.dma_start(xt_tiles[0][:, 2 * d2:2 * d2 + 2, :],
                            io["xt"][:, 2 * d2:2 * d2 + 2, ts(0, 512)])
    for d2 in range(NDT // 2):
        nc.sync.dma_start(wk_sb[:, 2 * d2:2 * d2 + 2], io["wkt"][:, 2 * d2:2 * d2 + 2])
        nc.scalar.dma_start(xt_tiles[1][:, 2 * d2:2 * d2 + 2, :],
                            io["xt"][:, 2 * d2:2 * d2 + 2, ts(1, 512)])
    for d4 in range(NDT // 4):
        nc.sync.dma_start(wv_sb[:, 4 * d4:4 * d4 + 4], io["wvt"][:, 4 * d4:4 * d4 + 4])
    nc.sync.dma_start(cos_sb[:], io["cos2"][:])
    nc.sync.dma_start(sin_sb[:], io["sin2"][:])
    ones32 = const.tile([P, 32], BF16)
    nc.any.memset(ones32[:], 1.0)
    # selector: picks one representative row per 32-row column group and
    # sums them (bcast matmul lhsT), zeroing the redundant copies.
    sel4 = const.tile([P, P], BF16)
    nc.any.memset(sel4[:], 0.0)
    for g in range(4):
        nc.any.memset(sel4[ds(32 * g, 1), :], 1.0)
    sel1 = const.tile([P, P], BF16)
    nc.any.memset(sel1[:], 0.0)
    nc.any.memset(sel1[ds(0, 1), :], 1.0)

    nc.sync.dma_start(patf_sb[:], io["patf"][:])
    nc.sync.dma_start(patc_sb[:], io["patc"][:])
    nc.sync.dma_start(wo_sb[:], io["wot"][:])

    q_sb = persist.tile([P, HPC, TOK], BF16)   # [parity*64+i, h, tok]
    k_sb = persist.tile([P, HPC, TOK], BF16)
    v_sb = persist.tile([P, TOK // P, EC], BF16)  # [t%128, t-tile, (h, j)]
    outT_sb = persist.tile([P, B * HPC, S], BF16)  # [j, pair, s]

    den_rows = {pi: {} for pi in range(B * HPC)}

    def emit_proj(tb):
        """q/k/v projections + RoPE for one 512-token block."""
        if tb in xt_tiles:
            xt_t = xt_tiles.pop(tb)
        else:
            xt_t = xt_pool.tile([P, NDT, 512], BF16, tag="xt")
            for d4 in range(NDT // 4):
                nc.sync.dma_start(xt_t[:, 4 * d4:4 * d4 + 4, :],
                                  io["xt"][:, 4 * d4:4 * d4 + 4, ts(tb, 512)])

        # RoPE operand broadcast: both heads in one DVE op via a 0-stride
        # broadcast of cos/sin over h; each projection's rope chain is
        # emitted right after its own eviction so the DVE finishes roped
        # q/k as early as possible (score LDWs wait on it).
        cos_b = cos_sb[:, ts(tb, 512)].unsqueeze(1).broadcast_to([P, HPC, 512])
        sin_b = sin_sb[:, ts(tb, 512)].unsqueeze(1).broadcast_to([P, HPC, 512])
        for w_sb, dst in ((wq_sb, q_sb), (wk_sb, k_sb)):
            qk_ps = psum_big.tile([P, HPC, 512], F32, tag="big", name="qk_ps")
            for h in range(HPC):
                for dt in range(NDT):
                    nc.tensor.matmul(
                        qk_ps[:, h, :], lhsT=w_sb[:, dt, h, :], rhs=xt_t[:, dt, :],
                        start=(dt == 0), stop=(dt == NDT - 1),
                    )
            nc.vector.tensor_copy(dst[:, :, ts(tb, 512)], qk_ps[:])
            swp = rope_pool.tile([P, HPC, 512], BF16, tag="swp")
            nc.sync.dma_start(swp[0:64, :, :], dst[64:128, :, ts(tb, 512)])
            nc.sync.dma_start(swp[64:128, :, :], dst[0:64, :, ts(tb, 512)])
            r1 = rope_pool.tile([P, HPC, 512], BF16, tag="r1")
            nc.vector.tensor_mul(r1[:], dst[:, :, ts(tb, 512)], cos_b)
            r2 = rope_pool.tile([P, HPC, 512], BF16, tag="r2")
            nc.vector.tensor_mul(r2[:], swp[:], sin_b)
            nc.vector.tensor_add(dst[:, :, ts(tb, 512)], r1[:], r2[:])

        v_ps = psum_big.tile([P, 4, EC], F32, tag="big", name="v_ps")
        for q4 in range(4):
            for dt in range(NDT):
                nc.tensor.matmul(
                    v_ps[:, q4, :], lhsT=xt_t[:, dt, ts(q4, P)], rhs=wv_sb[:, dt, :],
                    start=(dt == 0), stop=(dt == NDT - 1),
                )
        nc.vector.tensor_copy(v_sb[:, tb * 4:(tb + 1) * 4, :], v_ps[:])

    def emit_att(pi, m):
        """scores + exp + av/den matmuls for pair pi, 512-query block m."""
        b, h = divmod(pi, HPC)
        s_sl = ds(b * S + m * 512, 512)
        tlist = blocks[m]
        n_t = len(tlist)
        av_ps = psum_acc.tile([P, 512], F32, tag="acc")
        den4_ps = psum_den.tile([P, 512], F32, tag="den")
        # scores + exp first (PE runs ahead of ACT through the psum
        # pool); den/av matmuls afterwards so PE never waits on exp.
        # Diagonal tiles only compute the live [o:512] column slice.
        # Tiles are processed in pairs sharing a 2-bank psum tile so a pair
        # of full tiles needs a single (double-width) exp activation.
        exs = []  # per tile: (ex2 tile, half index)
        for p2 in range(n_t // 2):
            pair = tlist[2 * p2:2 * p2 + 2]
            sc_ps = psum_big.tile([P, 2, 512], F32, tag="big", name="sc_ps")
            ex2 = exp_pool.tile([P, 2, 512], BF16, tag="ex2")
            for hh, (j, o, fpid, cpid) in enumerate(pair):
                nc.tensor.matmul(
                    sc_ps[:, hh, ds(o, 512 - o)],
                    lhsT=k_sb[:, h, ds(b * S + j * P, P)],
                    rhs=q_sb[:, h, ds(b * S + m * 512 + o, 512 - o)],
                    start=True, stop=True,
                )
            if all(o == 0 for _, o, _, _ in pair):
                nc.scalar.activation(ex2[:], sc_ps[:], AF.Exp, scale=SCALE)
            else:
                for hh, (j, o, fpid, cpid) in enumerate(pair):
                    c_sl = ds(o, 512 - o)
                    nc.scalar.activation(ex2[:, hh, c_sl], sc_ps[:, hh, c_sl],
                                         AF.Exp, scale=SCALE)
            for hh, (j, o, fpid, cpid) in enumerate(pair):
                if fpid is not None:
                    nc.vector.tensor_mul(ex2[:, hh, :], ex2[:, hh, :],
                                         patf_sb[:, fpid, :])
                elif cpid is not None:
                    nc.vector.tensor_mul(ex2[:, hh, ds(o, P)],
                                         ex2[:, hh, ds(o, P)],
                                         patc_sb[:, cpid, :])
                exs.append((ex2, hh))
        # av matmuls per round of 4, then the 4 den matmuls issued
        # back-to-back into distinct PE column groups so they execute
        # concurrently (~1 matmul's time for all 4).
        single_group = (n_t == 4)
        nr = n_t // 4
        for r in range(nr):
            for q4 in range(4):
                idx = 4 * r + q4
                o = tlist[idx][1]
                c_sl = ds(o, 512 - o)
                ex2, hh = exs[idx]
                if single_group:
                    # sliced tiles may appear in the first round of 4 only
                    # if they all accumulate into one column group whose
                    # first (idx 0) matmul is full width
                    nc.tensor.matmul(
                        den4_ps[ds(0, 32), c_sl], lhsT=ones32[:],
                        rhs=ex2[:, hh, c_sl],
                        start=(idx == 0), stop=(idx == n_t - 1),
                    )
                else:
                    nc.tensor.matmul(
                        den4_ps[ds(32 * q4, 32), c_sl], lhsT=ones32[:],
                        rhs=ex2[:, hh, c_sl], start=(r == 0), stop=(r == nr - 1),
                        tile_position=(0, 32 * q4),
                    )
            for q4 in range(4):
                idx = 4 * r + q4
                j, o = tlist[idx][0], tlist[idx][1]
                c_sl = ds(o, 512 - o)
                ex2, hh = exs[idx]
                nc.tensor.matmul(
                    av_ps[:, c_sl], lhsT=v_sb[:, b * NTT + j, ds(h * HD, HD)],
                    rhs=ex2[:, hh, c_sl],
                    start=(idx == 0), stop=(idx == n_t - 1),
                )
        # evict the denominator; the av psum is held (psum_acc has 2 bufs)
        # until the staggered norm step, which folds the 1/den multiply into
        # the psum->sbuf eviction itself.
        den4_sb = den_pool.tile([P, 512], BF16, tag="denr")
        nc.vector.tensor_copy(den4_sb[:], den4_ps[:])
        den_rows[pi][m] = (den4_sb, sel1 if single_group else sel4, av_ps)

    def emit_norm(pi, m2):
        """broadcast den, reciprocal, then evict av psum * (1/den) -> outT."""
        d4, sel, av_ps = den_rows[pi].pop(m2)
        bc_ps = psum_bc.tile([P, 512], F32, tag="bc")
        nc.tensor.matmul(bc_ps[:], lhsT=sel[:], rhs=d4[:],
                         start=True, stop=True)
        rc = recip_pool.tile([P, 512], F32, tag="rc")
        nc.vector.reciprocal_approx_fast(rc[:], bc_ps[:])
        sl2 = ds(m2 * 512, 512)
        nc.vector.tensor_mul(outT_sb[:, pi, sl2], av_ps[:], rc[:])

    def emit_oproj(st):
        """o_proj partial y[s, e] = sum_h outT_h^T @ woT_h for one s-tile."""
        b = st // NTT
        sl = st % NTT
        for e2 in range(D // 1024):
            y_ps = psum_big.tile([P, 1024], F32, tag="big", name="y_ps")
            for i2 in range(2):
                for h in range(HPC):
                    nc.tensor.matmul(
                        y_ps[:, ds(i2 * 512, 512)],
                        lhsT=outT_sb[:, b * HPC + h, ts(sl, P)],
                        rhs=wo_sb[:, h, ts(e2 * 2 + i2, 512)],
                        start=(h == 0), stop=(h == HPC - 1),
                    )
            y_sb = y_pool.tile([P, 1024], BF16, tag="y")
            if (st * 2 + e2) % 2 == 0:
                nc.scalar.copy(y_sb[:], y_ps[:])
            else:
                nc.vector.tensor_copy(y_sb[:], y_ps[:])
            nc.sync.dma_start(io["y"][st, :, ts(e2, 1024)], y_sb[:])

    # Interleaved schedule: attention blocks (ACT-heavy: exp) are emitted
    # between projection / o_proj chunks (pure PE) so the in-order PE stream
    # always has matmul work while the scalar engine chews on exponentials.
    # Per-block norms are staggered one block behind their attention so the
    # chain (bcast -> recip -> mul) never gates the tail; o_proj tiles are
    # emitted progressively as soon as their s-block is normalized on both
    # heads (OPROJ(st) needs NORM(*, (st % 16) // 4)).
    emit_proj(0)
    emit_proj(1)
    emit_proj(2)
    emit_att(0, 0)
    emit_att(1, 0)
    emit_proj(3)
    emit_norm(0, 0)
    emit_norm(1, 0)
    emit_att(0, 1)
    emit_att(1, 1)
    emit_proj(4)
    emit_norm(0, 1)
    emit_norm(1, 1)
    emit_att(0, 2)
    emit_att(1, 2)
    emit_proj(5)
    emit_norm(0, 2)
    emit_norm(1, 2)
    emit_att(0, 3)
    emit_att(1, 3)
    emit_proj(6)
    emit_norm(0, 3)
    emit_norm(1, 3)
    emit_att(2, 1)
    emit_att(3, 1)
    emit_proj(7)
    emit_norm(2, 1)
    emit_norm(3, 1)
    emit_att(2, 2)
    emit_att(3, 2)
    for st in range(0, 4):
        emit_oproj(st)
    emit_norm(2, 2)
    emit_norm(3, 2)
    emit_att(2, 3)
    emit_att(3, 3)
    for st in range(4, 16):
        emit_oproj(st)
    emit_norm(2, 3)
    emit_norm(3, 3)
    emit_att(2, 0)
    emit_att(3, 0)
    for st in range(20, 32):
        emit_oproj(st)
    emit_norm(2, 0)
    emit_norm(3, 0)
    for st in range(16, 20):
        emit_oproj(st)


def _build_program(blocks_key, blocks, npatf, npatc):
    nc = bacc.Bacc(
        "TRN2", target_bir_lowering=False, debug=False, enable_asserts=False
    )
    io = {
        "xt": nc.dram_tensor("xt", [P, NDT, TOK], BF16, kind="ExternalInput").ap(),
        "wqt": nc.dram_tensor("wqt", [P, NDT, HPC, P], BF16, kind="ExternalInput").ap(),
        "wkt": nc.dram_tensor("wkt", [P, NDT, HPC, P], BF16, kind="ExternalInput").ap(),
        "wvt": nc.dram_tensor("wvt", [P, NDT, EC], BF16, kind="ExternalInput").ap(),
        "wot": nc.dram_tensor("wot", [P, HPC, D], BF16, kind="ExternalInput").ap(),
        "cos2": nc.dram_tensor("cos2", [P, TOK], BF16, kind="ExternalInput").ap(),
        "sin2": nc.dram_tensor("sin2", [P, TOK], BF16, kind="ExternalInput").ap(),
        "patf": nc.dram_tensor("patf", [P, npatf, 512], BF16, kind="ExternalInput").ap(),
        "patc": nc.dram_tensor("patc", [P, npatc, P], BF16, kind="ExternalInput").ap(),
        "y": nc.dram_tensor("y", [TOK // P, P, D], BF16, kind="ExternalOutput").ap(),
    }
    with tile.TileContext(nc) as tc:
        with ExitStack() as ctx:
            _emit(ctx, tc, io, blocks, npatf, npatc)
    nc.compile()
    return nc


def _get_program(mask):
    blocks, fpats, cpats = _classify_mask(mask)
    key = tuple(tuple(b) for b in blocks)
    if key not in _PROGRAM_CACHE:
        npatf = max(len(fpats), 1)
        npatc = max(len(cpats), 1)
        nc = _build_program(key, blocks, npatf, npatc)
        _PROGRAM_CACHE[key] = (nc, npatf, npatc)
    nc, npatf, npatc = _PROGRAM_CACHE[key]
    patf_np = np.zeros((P, npatf, 512), np.float32)
    for i, pt in enumerate(fpats):
        patf_np[:, i, :] = pt
    patc_np = np.zeros((P, npatc, P), np.float32)
    for i, pt in enumerate(cpats):
        patc_np[:, i, :] = pt
    return nc, patf_np, patc_np


def _bf16(a):
    return np.asarray(a, np.float32).astype(ml_dtypes.bfloat16)


def kernel(x, wq, wk, wv, wo, freqs_cos, freqs_sin, mask):
    global LAST_EXEC_NS
    x = np.asarray(x, np.float32)
    wq = np.asarray(wq, np.float32)
    wk = np.asarray(wk, np.float32)
    wv = np.asarray(wv, np.float32)
    wo = np.asarray(wo, np.float32)
    freqs_cos = np.asarray(freqs_cos, np.float32)
    freqs_sin = np.asarray(freqs_sin, np.float32)

    nc, patf_np, patc_np = _get_program(mask)

    # xT: [d, tok] -> [dp, dt, tok]
    xt = _bf16(
        x.reshape(TOK, D).T.reshape(NDT, P, TOK).transpose(1, 0, 2)
    )

    # cos/sin, parity-major RoPE operands: [128, tok]
    cosT = np.tile(freqs_cos.T, (1, B))          # [64, TOK]
    sinT = np.tile(freqs_sin.T, (1, B))
    cos2 = _bf16(np.concatenate([cosT, cosT], axis=0))
    sin2 = _bf16(np.concatenate([-sinT, sinT], axis=0))
    patf = _bf16(patf_np)
    patc = _bf16(patc_np)

    # per-head parity-major row permutation for q/k weights
    perm1 = np.r_[np.arange(0, P, 2), np.arange(1, P, 2)]

    in_maps = []
    for c in range(N_CORES):
        rows = slice(c * EC, (c + 1) * EC)
        wq_c, wk_c, wv_c = wq[rows], wk[rows], wv[rows]   # [256, D]
        wo_c = wo[:, rows]                                # [D, 256]
        row_perm = np.concatenate([h * P + perm1 for h in range(HPC)])
        wqt = _bf16(wq_c[row_perm].T.reshape(NDT, P, HPC, P).transpose(1, 0, 2, 3))
        wkt = _bf16(wk_c[row_perm].T.reshape(NDT, P, HPC, P).transpose(1, 0, 2, 3))
        wvt = _bf16(wv_c.T.reshape(NDT, P, EC).transpose(1, 0, 2))
        wot = _bf16(wo_c.T.reshape(HPC, P, D).transpose(1, 0, 2))
        in_maps.append({
            "xt": xt, "wqt": wqt, "wkt": wkt, "wvt": wvt, "wot": wot,
            "cos2": cos2, "sin2": sin2, "patf": patf, "patc": patc,
        })

    if BACKEND == "sim":
        from concourse.bass_interp import CoreSim
        results = []
        for c in range(N_CORES):
            sim = CoreSim(nc, trace=False)
            for name, arr in in_maps[c].items():
                sim.tensor(name)[:] = arr
            sim.tensor("y")[:] = 0
            sim.simulate()
            results.append({"y": np.array(sim.tensor("y"))})
    else:
        do_trace = TRACE and _install_trace_hook()
        res = run_bass_kernel_spmd(
            nc, in_maps, core_ids=list(range(N_CORES)), trace=do_trace,
        )
        results = res.results
        LAST_EXEC_NS = res.exec_time_ns

    y = np.zeros((TOK, D), np.float32)
    for c in range(N_CORES):
        y += results[c]["y"].reshape(TOK, D).astype(np.float32)
    return y.reshape(B, S, D)

